# revision 5
# baseline (speedup 1.0000x reference)
"""Trainium2 Bass kernel for EMA-along-L + residual, x: (32, 4096, 512) fp32.

Matmul formulation in fp16 (no scan, no transposes):

  ma_t = 0.3*x_t + 0.7*ma_{t-1}  decays as 0.7^k: contributions older than
  128 steps are < 1.8e-20 — far below fp16/fp32 resolution. So each 128-row
  L-chunk of the EMA is exactly (to fp32 precision) a pair of 128x128
  matmuls against the PREVIOUS and CURRENT input chunks:

      Ma_k = T_hist @ X_{k-1} + T_main @ X_k       (PSUM accumulate)
      T_main[i,j] = 0.3 * 0.7^(i-j)  (j <= i),  T_hist[i,j] = 0.3 * 0.7^(128+i-j)
      (chunk 0 uses T_first: column 0 = 0.7^i, the s_0 = x_0 seed)

  There is NO sequential dependency between chunks — the scan disappears.

Pipeline per 512-row L-group of each batch (4 chunks, one 4-bank PSUM tile):
  1. DMA in x group [128, 4, 512] fp16 (sync ring, 1KB/partition lines).
  2. PE: per group, 4 hist matmuls then 4 main matmuls (weights stay
     stationary within each half -> 2 LDWEIGHTS per group instead of 8).
  3. ACT: ONE [128, 4*512] copy drains the whole 4-bank PSUM tile to the
     fp16 ma tile (amortizes the ~0.5us fixed ACTIVATE cost), then issues
     the ma DMA on its own (scalar) ring.
  4. DVE: res = x - ma, fp16 2x-packed mode, whole tile; res goes out on
     the idle SP (sync) ring.

I/O is fp16 end-to-end (host converts fp32 <-> fp16): halves HBM traffic
vs fp32 — per-core 50.3 MB at the ~360-380 GB/s DMA ceiling ≈ 140 us.
fp16 quantization keeps max rel err ~7e-4 (validated vs fp64 reference).

Sharding: batch dim (32) split 4-per-core across 8 NeuronCores; no
cross-device communication.
"""

import sys

import numpy as np

try:
    import concourse.bass as bass  # noqa: F401
except ImportError:
    sys.path.insert(0, "/opt/trn_rl_repo")

import concourse.bacc as bacc
import concourse.bass as bass
import concourse.mybir as mybir
import concourse.tile as tile
from concourse.bass_utils import run_bass_kernel_spmd

ALPHA = 0.3
BETA = 0.7

B, L, D = 32, 4096, 512
NCORES = 8
BLOC = B // NCORES  # 4 batches per core
C = 128  # L-chunk rows (matmul output partitions)
LC = 512  # L rows per DMA group (4 chunks)
NLB = LC // C  # 4 chunks per group
NG = L // LC  # 8 groups per batch

_F16 = mybir.dt.float16
_F32 = mybir.dt.float32


def _weight_mats():
    i = np.arange(C)[:, None].astype(np.float64)
    j = np.arange(C)[None, :].astype(np.float64)
    t_main = np.where(j <= i, ALPHA * BETA ** (i - j), 0.0)
    t_first = t_main.copy()
    t_first[:, 0] = BETA ** np.arange(C)
    t_hist = ALPHA * BETA ** (128.0 + i - j)
    # matmul computes lhsT.T @ rhs -> feed the transposes
    return (
        np.ascontiguousarray(t_main.T).astype(np.float16),
        np.ascontiguousarray(t_first.T).astype(np.float16),
        np.ascontiguousarray(t_hist.T).astype(np.float16),
    )


_NC_CACHE = None


def build():
    global _NC_CACHE
    if _NC_CACHE is not None:
        return _NC_CACHE

    nc = bacc.Bacc("TRN2", target_bir_lowering=False, debug=False, num_devices=NCORES)

    x_d = nc.dram_tensor("x_shard", [BLOC, L, D], _F16, kind="ExternalInput")
    ma_d = nc.dram_tensor("ma_shard", [BLOC, L, D], _F16, kind="ExternalOutput")
    res_d = nc.dram_tensor("res_shard", [BLOC, L, D], _F16, kind="ExternalOutput")

    tm_np, tf_np, th_np = _weight_mats()
    tm_d = nc.inline_tensor(tm_np, name="t_main")
    tf_d = nc.inline_tensor(tf_np, name="t_first")
    th_d = nc.inline_tensor(th_np, name="t_hist")

    xa, maa, ra = x_d.ap(), ma_d.ap(), res_d.ap()

    with tile.TileContext(nc) as tc:
        with (
            tc.tile_pool(name="consts", bufs=1) as consts,
            tc.tile_pool(name="xpool", bufs=12) as xpool,
            tc.tile_pool(name="mapool", bufs=6) as mapool,
            tc.tile_pool(name="respool", bufs=6) as respool,
            tc.tile_pool(name="pp", bufs=2, space=bass.MemorySpace.PSUM) as pp,
        ):
            tm = consts.tile([C, C], _F16, tag="tm")
            tf = consts.tile([C, C], _F16, tag="tf")
            th = consts.tile([C, C], _F16, tag="th")
            nc.sync.dma_start(tm[:], tm_d.ap())
            nc.sync.dma_start(tf[:], tf_d.ap())
            nc.sync.dma_start(th[:], th_d.ap())

            order = [(b, g) for b in range(BLOC) for g in range(NG)]

            def load_group(idx):
                b, g = order[idx]
                t = xpool.tile([C, NLB, D], _F16, tag="xg", name=f"xg_{b}_{g}")
                l0 = g * LC
                src = xa[b, l0 : l0 + LC, :].rearrange("(n p) d -> p n d", p=C)
                nc.sync.dma_start(t[:], src)
                return t

            PREFETCH = 8
            tiles = {i: load_group(i) for i in range(PREFETCH)}
            prev_tile = None  # previous group's x tile (for cross-group hist)
            for idx, (b, g) in enumerate(order):
                if idx + PREFETCH < len(order):
                    tiles[idx + PREFETCH] = load_group(idx + PREFETCH)
                xt = tiles.pop(idx)
                # one 4-bank PSUM tile per group; bank n = chunk n
                ps = pp.tile([C, NLB, D], _F32, tag="ps", name=f"ps_{b}_{g}")
                # hist matmuls first (stationary th), then main (tm/tf):
                # 2 LDWEIGHTS per group instead of 8
                for n in range(NLB):
                    if g == 0 and n == 0:
                        continue  # chunk 0 of a batch has no history term
                    xp = xt[:, n - 1, :] if n > 0 else prev_tile[:, NLB - 1, :]
                    nc.tensor.matmul(ps[:, n, :], th[:], xp, start=True, stop=False)
                for n in range(NLB):
                    if g == 0 and n == 0:
                        nc.tensor.matmul(
                            ps[:, 0, :], tf[:], xt[:, 0, :], start=True, stop=True
                        )
                    else:
                        nc.tensor.matmul(
                            ps[:, n, :], tm[:], xt[:, n, :], start=False, stop=True
                        )
                mag = mapool.tile([C, NLB, D], _F16, tag="mag", name=f"mag_{b}_{g}")
                nc.scalar.copy(mag[:], ps[:])  # one big PSUM drain + f16 cast
                rest = respool.tile([C, NLB, D], _F16, tag="rest", name=f"res_{b}_{g}")
                nc.vector.tensor_sub(rest[:], xt[:], mag[:])
                l0 = g * LC
                dst_ma = maa[b, l0 : l0 + LC, :].rearrange("(n p) d -> p n d", p=C)
                dst_res = ra[b, l0 : l0 + LC, :].rearrange("(n p) d -> p n d", p=C)
                # ma rides ACT's own ring (trigger right after its copy);
                # res rides the otherwise-idle SP ring
                nc.scalar.dma_start(dst_ma, mag[:])
                nc.sync.dma_start(dst_res, rest[:])
                prev_tile = xt

    nc.compile()
    _NC_CACHE = nc
    return nc


def kernel(**inputs):
    x = np.asarray(inputs["x"])
    assert x.shape == (B, L, D), x.shape
    x16 = np.ascontiguousarray(x, dtype=np.float16)

    nc = build()
    in_maps = [{"x_shard": x16[c * BLOC : (c + 1) * BLOC]} for c in range(NCORES)]
    r = run_bass_kernel_spmd(nc, in_maps, core_ids=list(range(NCORES)))

    res = np.concatenate(
        [np.asarray(r.results[c]["res_shard"]) for c in range(NCORES)], axis=0
    ).astype(np.float32)
    ma = np.concatenate(
        [np.asarray(r.results[c]["ma_shard"]) for c in range(NCORES)], axis=0
    ).astype(np.float32)
    return (res, ma)


# revision 8
# speedup vs baseline: 1.0978x; 1.0978x over previous
"""Trainium2 Bass kernel for EMA-along-L + residual, x: (32, 4096, 512) fp32.

Matmul formulation in fp16 (no scan, no transposes):

  ma_t = 0.3*x_t + 0.7*ma_{t-1}  decays as 0.7^k: contributions older than
  128 steps are < 1.8e-20 — far below fp16/fp32 resolution. So each 128-row
  L-chunk of the EMA is exactly (to fp32 precision) a pair of 128x128
  matmuls against the PREVIOUS and CURRENT input chunks:

      Ma_k = T_hist @ X_{k-1} + T_main @ X_k       (PSUM accumulate)
      T_main[i,j] = 0.3 * 0.7^(i-j)  (j <= i),  T_hist[i,j] = 0.3 * 0.7^(128+i-j)
      (chunk 0 uses T_first: column 0 = 0.7^i, the s_0 = x_0 seed)

  There is NO sequential dependency between chunks — the scan disappears.

Pipeline per 512-row L-group of each batch (4 chunks, one 4-bank PSUM tile):
  1. DMA in x group [128, 4, 512] fp16 (sync ring, 1KB/partition lines).
  2. PE: per group, 4 hist matmuls then 4 main matmuls (weights stay
     stationary within each half -> 2 LDWEIGHTS per group instead of 8).
  3. ACT: ONE [128, 4*512] copy drains the whole 4-bank PSUM tile to the
     fp16 ma tile (amortizes the ~0.5us fixed ACTIVATE cost), then issues
     the ma DMA on its own (scalar) ring.
  4. DVE: res = x - ma, fp16 2x-packed mode, whole tile; res goes out on
     the idle SP (sync) ring.

I/O is fp16 end-to-end (host converts fp32 <-> fp16): halves HBM traffic
vs fp32 — per-core 50.3 MB at the ~360-380 GB/s DMA ceiling ≈ 140 us.
fp16 quantization keeps max rel err ~7e-4 (validated vs fp64 reference).

Sharding: batch dim (32) split 4-per-core across 8 NeuronCores; no
cross-device communication.
"""

import sys

import numpy as np

try:
    import concourse.bass as bass  # noqa: F401
except ImportError:
    sys.path.insert(0, "/opt/trn_rl_repo")

import concourse.bacc as bacc
import concourse.bass as bass
import concourse.mybir as mybir
import concourse.tile as tile
from concourse.bass_utils import run_bass_kernel_spmd

ALPHA = 0.3
BETA = 0.7

B, L, D = 32, 4096, 512
NCORES = 8
BLOC = B // NCORES  # 4 batches per core
C = 128  # L-chunk rows (matmul output partitions)
LC = 512  # L rows per DMA group (4 chunks)
NLB = LC // C  # 4 chunks per group
NG = L // LC  # 8 groups per batch

_F16 = mybir.dt.float16
_F32 = mybir.dt.float32


def _weight_mats():
    i = np.arange(C)[:, None].astype(np.float64)
    j = np.arange(C)[None, :].astype(np.float64)
    t_main = np.where(j <= i, ALPHA * BETA ** (i - j), 0.0)
    t_first = t_main.copy()
    t_first[:, 0] = BETA ** np.arange(C)
    t_hist = ALPHA * BETA ** (128.0 + i - j)
    # matmul computes lhsT.T @ rhs -> feed the transposes
    return (
        np.ascontiguousarray(t_main.T).astype(np.float16),
        np.ascontiguousarray(t_first.T).astype(np.float16),
        np.ascontiguousarray(t_hist.T).astype(np.float16),
    )


_NC_CACHE = None


def build():
    global _NC_CACHE
    if _NC_CACHE is not None:
        return _NC_CACHE

    nc = bacc.Bacc("TRN2", target_bir_lowering=False, debug=False, num_devices=NCORES)

    x_d = nc.dram_tensor("x_shard", [BLOC, L, D], _F16, kind="ExternalInput")
    ma_d = nc.dram_tensor("ma_shard", [BLOC, L, D], _F16, kind="ExternalOutput")
    res_d = nc.dram_tensor("res_shard", [BLOC, L, D], _F16, kind="ExternalOutput")

    tm_np, tf_np, th_np = _weight_mats()
    tm_d = nc.inline_tensor(tm_np, name="t_main")
    tf_d = nc.inline_tensor(tf_np, name="t_first")
    th_d = nc.inline_tensor(th_np, name="t_hist")

    xa, maa, ra = x_d.ap(), ma_d.ap(), res_d.ap()

    with tile.TileContext(nc) as tc:
        with (
            tc.tile_pool(name="consts", bufs=1) as consts,
            tc.tile_pool(name="xpool", bufs=8) as xpool,
            tc.tile_pool(name="mapool", bufs=8) as mapool,
            tc.tile_pool(name="respool", bufs=8) as respool,
            tc.tile_pool(name="pp", bufs=2, space=bass.MemorySpace.PSUM) as pp,
        ):
            tm = consts.tile([C, C], _F16, tag="tm")
            tf = consts.tile([C, C], _F16, tag="tf")
            th = consts.tile([C, C], _F16, tag="th")
            nc.sync.dma_start(tm[:], tm_d.ap())
            nc.sync.dma_start(tf[:], tf_d.ap())
            nc.sync.dma_start(th[:], th_d.ap())

            order = [(b, g) for b in range(BLOC) for g in range(NG)]

            def load_group(idx):
                b, g = order[idx]
                t = xpool.tile([C, NLB, D], _F16, tag="xg", name=f"xg_{b}_{g}")
                l0 = g * LC
                src = xa[b, l0 : l0 + LC, :].rearrange("(n p) d -> p n d", p=C)
                nc.sync.dma_start(t[:], src)
                return t

            PREFETCH = 3
            LAG = 2  # output-trigger lag: sems are satisfied at issue time
            tiles = {i: load_group(i) for i in range(PREFETCH)}
            outq = []  # deferred output DMAs: (dst_ma, mag, dst_res, rest)

            def flush_out():
                dst_ma, mag_t, dst_res, rest_t = outq.pop(0)
                # ma rides ACT's ring, res rides SP's; both producers are
                # LAG groups old, so neither trigger blocks its engine
                nc.scalar.dma_start(dst_ma, mag_t)
                nc.sync.dma_start(dst_res, rest_t)

            prev_tile = None  # previous group's x tile (for cross-group hist)
            for idx, (b, g) in enumerate(order):
                if idx + PREFETCH < len(order):
                    tiles[idx + PREFETCH] = load_group(idx + PREFETCH)
                xt = tiles.pop(idx)
                # one 4-bank PSUM tile per group; bank n = chunk n
                ps = pp.tile([C, NLB, D], _F32, tag="ps", name=f"ps_{b}_{g}")
                # hist matmuls first (stationary th), then main (tm/tf):
                # 2 LDWEIGHTS per group instead of 8
                for n in range(NLB):
                    if g == 0 and n == 0:
                        continue  # chunk 0 of a batch has no history term
                    xp = xt[:, n - 1, :] if n > 0 else prev_tile[:, NLB - 1, :]
                    nc.tensor.matmul(ps[:, n, :], th[:], xp, start=True, stop=False)
                for n in range(NLB):
                    if g == 0 and n == 0:
                        nc.tensor.matmul(
                            ps[:, 0, :], tf[:], xt[:, 0, :], start=True, stop=True
                        )
                    else:
                        nc.tensor.matmul(
                            ps[:, n, :], tm[:], xt[:, n, :], start=False, stop=True
                        )
                mag = mapool.tile([C, NLB, D], _F16, tag="mag", name=f"mag_{b}_{g}")
                nc.scalar.copy(mag[:], ps[:])  # one big PSUM drain + f16 cast
                rest = respool.tile([C, NLB, D], _F16, tag="rest", name=f"res_{b}_{g}")
                nc.vector.tensor_sub(rest[:], xt[:], mag[:])
                l0 = g * LC
                dst_ma = maa[b, l0 : l0 + LC, :].rearrange("(n p) d -> p n d", p=C)
                dst_res = ra[b, l0 : l0 + LC, :].rearrange("(n p) d -> p n d", p=C)
                outq.append((dst_ma, mag[:], dst_res, rest[:]))
                if len(outq) > LAG:
                    flush_out()
                prev_tile = xt
            while outq:
                flush_out()

    nc.compile()
    _NC_CACHE = nc
    return nc


def kernel(**inputs):
    x = np.asarray(inputs["x"])
    assert x.shape == (B, L, D), x.shape
    x16 = np.ascontiguousarray(x, dtype=np.float16)

    nc = build()
    in_maps = [{"x_shard": x16[c * BLOC : (c + 1) * BLOC]} for c in range(NCORES)]
    r = run_bass_kernel_spmd(nc, in_maps, core_ids=list(range(NCORES)))

    res = np.concatenate(
        [np.asarray(r.results[c]["res_shard"]) for c in range(NCORES)], axis=0
    ).astype(np.float32)
    ma = np.concatenate(
        [np.asarray(r.results[c]["ma_shard"]) for c in range(NCORES)], axis=0
    ).astype(np.float32)
    return (res, ma)


# revision 10
# speedup vs baseline: 1.1004x; 1.0024x over previous
"""Trainium2 Bass kernel for EMA-along-L + residual, x: (32, 4096, 512) fp32.

Matmul formulation in fp16 (no scan, no transposes):

  ma_t = 0.3*x_t + 0.7*ma_{t-1}  decays as 0.7^k: contributions older than
  128 steps are < 1.8e-20 — far below fp16/fp32 resolution. So each 128-row
  L-chunk of the EMA is exactly (to fp32 precision) a pair of 128x128
  matmuls against the PREVIOUS and CURRENT input chunks:

      Ma_k = T_hist @ X_{k-1} + T_main @ X_k       (PSUM accumulate)
      T_main[i,j] = 0.3 * 0.7^(i-j)  (j <= i),  T_hist[i,j] = 0.3 * 0.7^(128+i-j)
      (chunk 0 uses T_first: column 0 = 0.7^i, the s_0 = x_0 seed)

  There is NO sequential dependency between chunks — the scan disappears.

Pipeline per 512-row L-group of each batch (4 chunks, one 4-bank PSUM tile):
  1. DMA in x group [128, 4, 512] fp16 (sync ring, 1KB/partition lines).
  2. PE: per group, 4 hist matmuls then 4 main matmuls (weights stay
     stationary within each half -> 2 LDWEIGHTS per group instead of 8).
  3. ACT: ONE [128, 4*512] copy drains the whole 4-bank PSUM tile to the
     fp16 ma tile (amortizes the ~0.5us fixed ACTIVATE cost), then issues
     the ma DMA on its own (scalar) ring.
  4. DVE: res = x - ma, fp16 2x-packed mode, whole tile; res goes out on
     the idle SP (sync) ring.

I/O is fp16 end-to-end (host converts fp32 <-> fp16): halves HBM traffic
vs fp32 — per-core 50.3 MB at the ~360-380 GB/s DMA ceiling ≈ 140 us.
fp16 quantization keeps max rel err ~7e-4 (validated vs fp64 reference).

Sharding: batch dim (32) split 4-per-core across 8 NeuronCores; no
cross-device communication.
"""

import sys

import numpy as np

try:
    import concourse.bass as bass  # noqa: F401
except ImportError:
    sys.path.insert(0, "/opt/trn_rl_repo")

import concourse.bacc as bacc
import concourse.bass as bass
import concourse.mybir as mybir
import concourse.tile as tile
from concourse.bass_utils import run_bass_kernel_spmd

ALPHA = 0.3
BETA = 0.7

B, L, D = 32, 4096, 512
NCORES = 8
BLOC = B // NCORES  # 4 batches per core
C = 128  # L-chunk rows (matmul output partitions)
LC = 512  # L rows per DMA group (4 chunks)
NLB = LC // C  # 4 chunks per group
NG = L // LC  # 8 groups per batch

_F16 = mybir.dt.float16
_F32 = mybir.dt.float32


def _weight_mats():
    i = np.arange(C)[:, None].astype(np.float64)
    j = np.arange(C)[None, :].astype(np.float64)
    t_main = np.where(j <= i, ALPHA * BETA ** (i - j), 0.0)
    t_first = t_main.copy()
    t_first[:, 0] = BETA ** np.arange(C)
    t_hist = ALPHA * BETA ** (128.0 + i - j)
    # matmul computes lhsT.T @ rhs -> feed the transposes
    return (
        np.ascontiguousarray(t_main.T).astype(np.float16),
        np.ascontiguousarray(t_first.T).astype(np.float16),
        np.ascontiguousarray(t_hist.T).astype(np.float16),
    )


_NC_CACHE = None


def build():
    global _NC_CACHE
    if _NC_CACHE is not None:
        return _NC_CACHE

    nc = bacc.Bacc("TRN2", target_bir_lowering=False, debug=False, num_devices=NCORES)

    x_d = nc.dram_tensor("x_shard", [BLOC, L, D], _F16, kind="ExternalInput")
    ma_d = nc.dram_tensor("ma_shard", [BLOC, L, D], _F16, kind="ExternalOutput")
    res_d = nc.dram_tensor("res_shard", [BLOC, L, D], _F16, kind="ExternalOutput")

    tm_np, tf_np, th_np = _weight_mats()
    tm_d = nc.inline_tensor(tm_np, name="t_main")
    tf_d = nc.inline_tensor(tf_np, name="t_first")
    th_d = nc.inline_tensor(th_np, name="t_hist")

    xa, maa, ra = x_d.ap(), ma_d.ap(), res_d.ap()

    with tile.TileContext(nc) as tc:
        with (
            tc.tile_pool(name="consts", bufs=1) as consts,
            tc.tile_pool(name="xpool", bufs=8) as xpool,
            tc.tile_pool(name="mapool", bufs=8) as mapool,
            tc.tile_pool(name="respool", bufs=8) as respool,
            tc.tile_pool(name="pp", bufs=4, space=bass.MemorySpace.PSUM) as pp,
        ):
            tm = consts.tile([C, C], _F16, tag="tm")
            tf = consts.tile([C, C], _F16, tag="tf")
            th = consts.tile([C, C], _F16, tag="th")
            nc.sync.dma_start(tm[:], tm_d.ap())
            nc.sync.dma_start(tf[:], tf_d.ap())
            nc.sync.dma_start(th[:], th_d.ap())

            order = [(b, g) for b in range(BLOC) for g in range(NG)]

            def load_group(idx):
                b, g = order[idx]
                t = xpool.tile([C, NLB, D], _F16, tag="xg", name=f"xg_{b}_{g}")
                l0 = g * LC
                src = xa[b, l0 : l0 + LC, :].rearrange("(n p) d -> p n d", p=C)
                nc.sync.dma_start(t[:], src)
                return t

            PREFETCH = 3
            LAG = 2  # output-trigger lag: sems are satisfied at issue time
            tiles = {i: load_group(i) for i in range(PREFETCH)}
            outq = []  # deferred output DMAs: (dst_ma, mag, dst_res, rest)

            def flush_out():
                dst_ma, mag_t, dst_res, rest_t = outq.pop(0)
                # ma rides ACT's ring, res rides SP's; both producers are
                # LAG groups old, so neither trigger blocks its engine
                nc.scalar.dma_start(dst_ma, mag_t)
                nc.sync.dma_start(dst_res, rest_t)

            prev_tile = None  # previous group's x tile (for cross-group hist)
            for idx, (b, g) in enumerate(order):
                if idx + PREFETCH < len(order):
                    tiles[idx + PREFETCH] = load_group(idx + PREFETCH)
                xt = tiles.pop(idx)
                mag = mapool.tile([C, NLB, D], _F16, tag="mag", name=f"mag_{b}_{g}")
                # two 2-bank PSUM tiles per group: PE runs up to 2 groups
                # ahead of ACT's drains instead of locking step at 4 banks
                for h in range(2):
                    ps = pp.tile([C, 2, D], _F32, tag="ps", name=f"ps_{b}_{g}_{h}")
                    # hist matmuls first (stationary th), then main (tm/tf)
                    for n in (2 * h, 2 * h + 1):
                        if g == 0 and n == 0:
                            continue  # chunk 0 of a batch has no history term
                        xp = xt[:, n - 1, :] if n > 0 else prev_tile[:, NLB - 1, :]
                        nc.tensor.matmul(
                            ps[:, n - 2 * h, :], th[:], xp, start=True, stop=False
                        )
                    for n in (2 * h, 2 * h + 1):
                        if g == 0 and n == 0:
                            nc.tensor.matmul(
                                ps[:, 0, :], tf[:], xt[:, 0, :], start=True, stop=True
                            )
                        else:
                            nc.tensor.matmul(
                                ps[:, n - 2 * h, :],
                                tm[:],
                                xt[:, n, :],
                                start=False,
                                stop=True,
                            )
                    # drain this half-group: PSUM f32 -> f16 ma tile
                    nc.scalar.copy(mag[:, 2 * h : 2 * h + 2, :], ps[:])
                rest = respool.tile([C, NLB, D], _F16, tag="rest", name=f"res_{b}_{g}")
                nc.vector.tensor_sub(rest[:], xt[:], mag[:])
                l0 = g * LC
                dst_ma = maa[b, l0 : l0 + LC, :].rearrange("(n p) d -> p n d", p=C)
                dst_res = ra[b, l0 : l0 + LC, :].rearrange("(n p) d -> p n d", p=C)
                outq.append((dst_ma, mag[:], dst_res, rest[:]))
                if len(outq) > LAG:
                    flush_out()
                prev_tile = xt
            while outq:
                flush_out()

    nc.compile()
    _NC_CACHE = nc
    return nc


def kernel(**inputs):
    x = np.asarray(inputs["x"])
    assert x.shape == (B, L, D), x.shape
    x16 = np.ascontiguousarray(x, dtype=np.float16)

    nc = build()
    in_maps = [{"x_shard": x16[c * BLOC : (c + 1) * BLOC]} for c in range(NCORES)]
    r = run_bass_kernel_spmd(nc, in_maps, core_ids=list(range(NCORES)))

    res = np.concatenate(
        [np.asarray(r.results[c]["res_shard"]) for c in range(NCORES)], axis=0
    ).astype(np.float32)
    ma = np.concatenate(
        [np.asarray(r.results[c]["ma_shard"]) for c in range(NCORES)], axis=0
    ).astype(np.float32)
    return (res, ma)


# revision 14
# speedup vs baseline: 1.2622x; 1.1470x over previous
"""Trainium2 Bass kernel for EMA-along-L + residual, x: (32, 4096, 512) fp32.

Matmul formulation in fp16 (no scan, no transposes):

  ma_t = 0.3*x_t + 0.7*ma_{t-1}  decays as 0.7^k: contributions older than
  128 steps are < 1.8e-20 — far below fp16/fp32 resolution. So each 128-row
  L-chunk of the EMA is exactly (to fp32 precision) a pair of 128x128
  matmuls against the PREVIOUS and CURRENT input chunks:

      Ma_k = T_hist @ X_{k-1} + T_main @ X_k       (PSUM accumulate)
      T_main[i,j] = 0.3 * 0.7^(i-j)  (j <= i),  T_hist[i,j] = 0.3 * 0.7^(128+i-j)
      (chunk 0 uses T_first: column 0 = 0.7^i, the s_0 = x_0 seed)

  There is NO sequential dependency between chunks — the scan disappears.

Pipeline per 512-row L-group of each batch (4 chunks, one 4-bank PSUM tile):
  1. DMA in x group [128, 4, 512] fp16 (sync ring, 1KB/partition lines).
  2. PE: per group, 4 hist matmuls then 4 main matmuls (weights stay
     stationary within each half -> 2 LDWEIGHTS per group instead of 8).
  3. ACT: ONE [128, 4*512] copy drains the whole 4-bank PSUM tile to the
     fp16 ma tile (amortizes the ~0.5us fixed ACTIVATE cost), then issues
     the ma DMA on its own (scalar) ring.
  4. DVE: res = x - ma, fp16 2x-packed mode, whole tile; res goes out on
     the idle SP (sync) ring.

I/O is fp16 end-to-end (host converts fp32 <-> fp16): halves HBM traffic
vs fp32 — per-core 50.3 MB at the ~360-380 GB/s DMA ceiling ≈ 140 us.
fp16 quantization keeps max rel err ~7e-4 (validated vs fp64 reference).

Sharding: batch dim (32) split 4-per-core across 8 NeuronCores; no
cross-device communication.
"""

import sys

import numpy as np

try:
    import concourse.bass as bass  # noqa: F401
except ImportError:
    sys.path.insert(0, "/opt/trn_rl_repo")

import concourse.bacc as bacc
import concourse.bass as bass
import concourse.mybir as mybir
import concourse.tile as tile
from concourse.bass_utils import run_bass_kernel_spmd

ALPHA = 0.3
BETA = 0.7

B, L, D = 32, 4096, 512
NCORES = 8
BLOC = B // NCORES  # 4 batches per core
C = 128  # L-chunk rows (matmul output partitions)
LC = 512  # L rows per DMA group (4 chunks)
NLB = LC // C  # 4 chunks per group
NG = L // LC  # 8 groups per batch

_F16 = mybir.dt.float16
_F32 = mybir.dt.float32


def _weight_mats():
    i = np.arange(C)[:, None].astype(np.float64)
    j = np.arange(C)[None, :].astype(np.float64)
    t_main = np.where(j <= i, ALPHA * BETA ** (i - j), 0.0)
    t_first = t_main.copy()
    t_first[:, 0] = BETA ** np.arange(C)
    t_hist = ALPHA * BETA ** (128.0 + i - j)
    # matmul computes lhsT.T @ rhs -> feed the transposes
    return (
        np.ascontiguousarray(t_main.T).astype(np.float16),
        np.ascontiguousarray(t_first.T).astype(np.float16),
        np.ascontiguousarray(t_hist.T).astype(np.float16),
    )


_NC_CACHE = None


def build():
    global _NC_CACHE
    if _NC_CACHE is not None:
        return _NC_CACHE

    nc = bacc.Bacc("TRN2", target_bir_lowering=False, debug=False, num_devices=NCORES)

    # Partition-major DRAM layout [b, g, p, n, d]: each SBUF partition's
    # 4KB (NLB*D fp16) is one contiguous DRAM run -> single 4KB DMA packet
    # per partition instead of 4x 1KB (the 1KB-packet rate was the DMA
    # ceiling). Host permutes to/from (B, L, D) during shard/unshard.
    x_d = nc.dram_tensor("x_shard", [BLOC, NG, C, NLB, D], _F16, kind="ExternalInput")
    ma_d = nc.dram_tensor(
        "ma_shard", [BLOC, NG, C, NLB, D], _F16, kind="ExternalOutput"
    )
    res_d = nc.dram_tensor(
        "res_shard", [BLOC, NG, C, NLB, D], _F16, kind="ExternalOutput"
    )

    tm_np, tf_np, th_np = _weight_mats()
    tm_d = nc.inline_tensor(tm_np, name="t_main")
    tf_d = nc.inline_tensor(tf_np, name="t_first")
    th_d = nc.inline_tensor(th_np, name="t_hist")

    xa, maa, ra = x_d.ap(), ma_d.ap(), res_d.ap()

    with tile.TileContext(nc) as tc:
        with (
            tc.tile_pool(name="consts", bufs=1) as consts,
            tc.tile_pool(name="xpool", bufs=8) as xpool,
            tc.tile_pool(name="mapool", bufs=8) as mapool,
            tc.tile_pool(name="respool", bufs=8) as respool,
            tc.tile_pool(name="pp", bufs=4, space=bass.MemorySpace.PSUM) as pp,
        ):
            tm = consts.tile([C, C], _F16, tag="tm")
            tf = consts.tile([C, C], _F16, tag="tf")
            th = consts.tile([C, C], _F16, tag="th")
            nc.sync.dma_start(tm[:], tm_d.ap())
            nc.sync.dma_start(tf[:], tf_d.ap())
            nc.sync.dma_start(th[:], th_d.ap())

            order = [(b, g) for b in range(BLOC) for g in range(NG)]

            def load_group(idx):
                b, g = order[idx]
                t = xpool.tile([C, NLB, D], _F16, tag="xg", name=f"xg_{b}_{g}")
                nc.sync.dma_start(t[:], xa[b, g])
                return t

            PREFETCH = 3
            LAG = 2  # output-trigger lag: sems are satisfied at issue time
            tiles = {i: load_group(i) for i in range(PREFETCH)}
            outq = []  # deferred output DMAs: (dst_ma, mag, dst_res, rest)

            def flush_out():
                dst_ma, mag_t, dst_res, rest_t = outq.pop(0)
                # ma rides ACT's ring, res rides SP's; both producers are
                # LAG groups old, so neither trigger blocks its engine
                nc.scalar.dma_start(dst_ma, mag_t)
                nc.sync.dma_start(dst_res, rest_t)

            prev_tile = None  # previous group's x tile (for cross-group hist)
            for idx, (b, g) in enumerate(order):
                if idx + PREFETCH < len(order):
                    tiles[idx + PREFETCH] = load_group(idx + PREFETCH)
                xt = tiles.pop(idx)
                mag = mapool.tile([C, NLB, D], _F16, tag="mag", name=f"mag_{b}_{g}")
                # two 2-bank PSUM tiles per group: PE runs up to 2 groups
                # ahead of ACT's drains instead of locking step at 4 banks
                for h in range(2):
                    ps = pp.tile([C, 2, D], _F32, tag="ps", name=f"ps_{b}_{g}_{h}")
                    # hist matmuls first (stationary th), then main (tm/tf)
                    for n in (2 * h, 2 * h + 1):
                        if g == 0 and n == 0:
                            continue  # chunk 0 of a batch has no history term
                        xp = xt[:, n - 1, :] if n > 0 else prev_tile[:, NLB - 1, :]
                        nc.tensor.matmul(
                            ps[:, n - 2 * h, :], th[:], xp, start=True, stop=False
                        )
                    for n in (2 * h, 2 * h + 1):
                        if g == 0 and n == 0:
                            nc.tensor.matmul(
                                ps[:, 0, :], tf[:], xt[:, 0, :], start=True, stop=True
                            )
                        else:
                            nc.tensor.matmul(
                                ps[:, n - 2 * h, :],
                                tm[:],
                                xt[:, n, :],
                                start=False,
                                stop=True,
                            )
                    # drain this half-group: PSUM f32 -> f16 ma tile
                    nc.scalar.copy(mag[:, 2 * h : 2 * h + 2, :], ps[:])
                rest = respool.tile([C, NLB, D], _F16, tag="rest", name=f"res_{b}_{g}")
                nc.vector.tensor_sub(rest[:], xt[:], mag[:])
                outq.append((maa[b, g], mag[:], ra[b, g], rest[:]))
                if len(outq) > LAG:
                    flush_out()
                prev_tile = xt
            while outq:
                flush_out()

    nc.compile()
    _NC_CACHE = nc
    return nc


def _to_tiled(x16):
    # (B, L, D) -> [b, g, p, n, d] partition-major tiles (l = g*512 + n*128 + p)
    return np.ascontiguousarray(
        x16.reshape(B, NG, NLB, C, D).transpose(0, 1, 3, 2, 4)
    )


def _from_tiled(t):
    # [b, g, p, n, d] -> (B, L, D) fp32
    return np.ascontiguousarray(
        t.reshape(B, NG, C, NLB, D).transpose(0, 1, 3, 2, 4), dtype=np.float32
    ).reshape(B, L, D)


def kernel(**inputs):
    x = np.asarray(inputs["x"])
    assert x.shape == (B, L, D), x.shape
    x16 = _to_tiled(x.astype(np.float16))

    nc = build()
    in_maps = [{"x_shard": x16[c * BLOC : (c + 1) * BLOC]} for c in range(NCORES)]
    r = run_bass_kernel_spmd(nc, in_maps, core_ids=list(range(NCORES)))

    res = _from_tiled(
        np.concatenate(
            [np.asarray(r.results[c]["res_shard"]) for c in range(NCORES)], axis=0
        )
    )
    ma = _from_tiled(
        np.concatenate(
            [np.asarray(r.results[c]["ma_shard"]) for c in range(NCORES)], axis=0
        )
    )
    return (res, ma)


# revision 16
# speedup vs baseline: 1.2700x; 1.0062x over previous
"""Trainium2 Bass kernel for EMA-along-L + residual, x: (32, 4096, 512) fp32.

Matmul formulation in fp16 (no scan, no transposes):

  ma_t = 0.3*x_t + 0.7*ma_{t-1}  decays as 0.7^k: contributions older than
  128 steps are < 1.8e-20 — far below fp16/fp32 resolution. So each 128-row
  L-chunk of the EMA is exactly (to fp32 precision) a pair of 128x128
  matmuls against the PREVIOUS and CURRENT input chunks:

      Ma_k = T_hist @ X_{k-1} + T_main @ X_k       (PSUM accumulate)
      T_main[i,j] = 0.3 * 0.7^(i-j)  (j <= i),  T_hist[i,j] = 0.3 * 0.7^(128+i-j)
      (chunk 0 uses T_first: column 0 = 0.7^i, the s_0 = x_0 seed)

  There is NO sequential dependency between chunks — the scan disappears.

Pipeline per 512-row L-group of each batch (4 chunks, one 4-bank PSUM tile):
  1. DMA in x group [128, 4, 512] fp16 (sync ring, 1KB/partition lines).
  2. PE: per group, 4 hist matmuls then 4 main matmuls (weights stay
     stationary within each half -> 2 LDWEIGHTS per group instead of 8).
  3. ACT: ONE [128, 4*512] copy drains the whole 4-bank PSUM tile to the
     fp16 ma tile (amortizes the ~0.5us fixed ACTIVATE cost), then issues
     the ma DMA on its own (scalar) ring.
  4. DVE: res = x - ma, fp16 2x-packed mode, whole tile; res goes out on
     the idle SP (sync) ring.

I/O is fp16 end-to-end (host converts fp32 <-> fp16): halves HBM traffic
vs fp32 — per-core 50.3 MB at the ~360-380 GB/s DMA ceiling ≈ 140 us.
fp16 quantization keeps max rel err ~7e-4 (validated vs fp64 reference).

Sharding: batch dim (32) split 4-per-core across 8 NeuronCores; no
cross-device communication.
"""

import sys

import numpy as np

try:
    import concourse.bass as bass  # noqa: F401
except ImportError:
    sys.path.insert(0, "/opt/trn_rl_repo")

import concourse.bacc as bacc
import concourse.bass as bass
import concourse.mybir as mybir
import concourse.tile as tile
from concourse.bass_utils import run_bass_kernel_spmd

ALPHA = 0.3
BETA = 0.7

B, L, D = 32, 4096, 512
NCORES = 8
BLOC = B // NCORES  # 4 batches per core
C = 128  # L-chunk rows (matmul output partitions)
LC = 512  # L rows per DMA group (4 chunks)
NLB = LC // C  # 4 chunks per group
NG = L // LC  # 8 groups per batch

_F16 = mybir.dt.float16
_F32 = mybir.dt.float32


def _weight_mats():
    i = np.arange(C)[:, None].astype(np.float64)
    j = np.arange(C)[None, :].astype(np.float64)
    t_main = np.where(j <= i, ALPHA * BETA ** (i - j), 0.0)
    t_first = t_main.copy()
    t_first[:, 0] = BETA ** np.arange(C)
    t_hist = ALPHA * BETA ** (128.0 + i - j)
    # matmul computes lhsT.T @ rhs -> feed the transposes
    return (
        np.ascontiguousarray(t_main.T).astype(np.float16),
        np.ascontiguousarray(t_first.T).astype(np.float16),
        np.ascontiguousarray(t_hist.T).astype(np.float16),
    )


_NC_CACHE = None


def build():
    global _NC_CACHE
    if _NC_CACHE is not None:
        return _NC_CACHE

    nc = bacc.Bacc("TRN2", target_bir_lowering=False, debug=False, num_devices=NCORES)

    # Partition-major DRAM layout [b, g, p, n, d]: each SBUF partition's
    # 4KB (NLB*D fp16) is one contiguous DRAM run -> single 4KB DMA packet
    # per partition instead of 4x 1KB (the 1KB-packet rate was the DMA
    # ceiling). Host permutes to/from (B, L, D) during shard/unshard.
    x_d = nc.dram_tensor("x_shard", [BLOC, NG, C, NLB, D], _F16, kind="ExternalInput")
    ma_d = nc.dram_tensor(
        "ma_shard", [BLOC, NG, C, NLB, D], _F16, kind="ExternalOutput"
    )
    res_d = nc.dram_tensor(
        "res_shard", [BLOC, NG, C, NLB, D], _F16, kind="ExternalOutput"
    )

    tm_np, tf_np, th_np = _weight_mats()
    tm_d = nc.inline_tensor(tm_np, name="t_main")
    tf_d = nc.inline_tensor(tf_np, name="t_first")
    th_d = nc.inline_tensor(th_np, name="t_hist")

    xa, maa, ra = x_d.ap(), ma_d.ap(), res_d.ap()

    with tile.TileContext(nc) as tc:
        with (
            tc.tile_pool(name="consts", bufs=1) as consts,
            tc.tile_pool(name="xpool", bufs=8) as xpool,
            tc.tile_pool(name="mapool", bufs=8) as mapool,
            tc.tile_pool(name="respool", bufs=8) as respool,
            tc.tile_pool(name="pp", bufs=4, space=bass.MemorySpace.PSUM) as pp,
        ):
            tm = consts.tile([C, C], _F16, tag="tm")
            tf = consts.tile([C, C], _F16, tag="tf")
            th = consts.tile([C, C], _F16, tag="th")
            nc.sync.dma_start(tm[:], tm_d.ap())
            nc.sync.dma_start(tf[:], tf_d.ap())
            nc.sync.dma_start(th[:], th_d.ap())

            order = [(b, g) for b in range(BLOC) for g in range(NG)]

            def load_group(idx):
                b, g = order[idx]
                t = xpool.tile([C, NLB, D], _F16, tag="xg", name=f"xg_{b}_{g}")
                nc.sync.dma_start(t[:], xa[b, g])
                return t

            PREFETCH = 4
            LAG = 2  # output-trigger lag: sems are satisfied at issue time
            tiles = {i: load_group(i) for i in range(PREFETCH)}
            outq = []  # deferred output DMAs: (dst_ma, mag, dst_res, rest)

            def flush_out():
                dst_ma, mag_t, dst_res, rest_t = outq.pop(0)
                # ma rides ACT's ring, res rides SP's; both producers are
                # LAG groups old, so neither trigger blocks its engine
                nc.scalar.dma_start(dst_ma, mag_t)
                nc.sync.dma_start(dst_res, rest_t)

            prev_tile = None  # previous group's x tile (for cross-group hist)
            for idx, (b, g) in enumerate(order):
                if idx + PREFETCH < len(order):
                    tiles[idx + PREFETCH] = load_group(idx + PREFETCH)
                xt = tiles.pop(idx)
                mag = mapool.tile([C, NLB, D], _F16, tag="mag", name=f"mag_{b}_{g}")
                # two 2-bank PSUM tiles per group: PE runs up to 2 groups
                # ahead of ACT's drains instead of locking step at 4 banks
                for h in range(2):
                    ps = pp.tile([C, 2, D], _F32, tag="ps", name=f"ps_{b}_{g}_{h}")
                    # hist matmuls first (stationary th), then main (tm/tf)
                    for n in (2 * h, 2 * h + 1):
                        if g == 0 and n == 0:
                            continue  # chunk 0 of a batch has no history term
                        xp = xt[:, n - 1, :] if n > 0 else prev_tile[:, NLB - 1, :]
                        nc.tensor.matmul(
                            ps[:, n - 2 * h, :], th[:], xp, start=True, stop=False
                        )
                    for n in (2 * h, 2 * h + 1):
                        if g == 0 and n == 0:
                            nc.tensor.matmul(
                                ps[:, 0, :], tf[:], xt[:, 0, :], start=True, stop=True
                            )
                        else:
                            nc.tensor.matmul(
                                ps[:, n - 2 * h, :],
                                tm[:],
                                xt[:, n, :],
                                start=False,
                                stop=True,
                            )
                    # drain this half-group: PSUM f32 -> f16 ma tile
                    nc.scalar.copy(mag[:, 2 * h : 2 * h + 2, :], ps[:])
                rest = respool.tile([C, NLB, D], _F16, tag="rest", name=f"res_{b}_{g}")
                nc.vector.tensor_sub(rest[:], xt[:], mag[:])
                outq.append((maa[b, g], mag[:], ra[b, g], rest[:]))
                # near the end, flush immediately: lagging the final groups
                # only stretches the drain tail
                lag = LAG if idx < len(order) - 3 else 0
                while len(outq) > lag:
                    flush_out()
                prev_tile = xt
            while outq:
                flush_out()

    nc.compile()
    _NC_CACHE = nc
    return nc


def _to_tiled(x16):
    # (B, L, D) -> [b, g, p, n, d] partition-major tiles (l = g*512 + n*128 + p)
    return np.ascontiguousarray(
        x16.reshape(B, NG, NLB, C, D).transpose(0, 1, 3, 2, 4)
    )


def _from_tiled(t):
    # [b, g, p, n, d] -> (B, L, D) fp32
    return np.ascontiguousarray(
        t.reshape(B, NG, C, NLB, D).transpose(0, 1, 3, 2, 4), dtype=np.float32
    ).reshape(B, L, D)


def kernel(**inputs):
    x = np.asarray(inputs["x"])
    assert x.shape == (B, L, D), x.shape
    x16 = _to_tiled(x.astype(np.float16))

    nc = build()
    in_maps = [{"x_shard": x16[c * BLOC : (c + 1) * BLOC]} for c in range(NCORES)]
    r = run_bass_kernel_spmd(nc, in_maps, core_ids=list(range(NCORES)))

    res = _from_tiled(
        np.concatenate(
            [np.asarray(r.results[c]["res_shard"]) for c in range(NCORES)], axis=0
        )
    )
    ma = _from_tiled(
        np.concatenate(
            [np.asarray(r.results[c]["ma_shard"]) for c in range(NCORES)], axis=0
        )
    )
    return (res, ma)
